# revision 16
# baseline (speedup 1.0000x reference)
"""Trainium2 Bass kernel for 2-layer LSTM (H=64) + linear head — v3.

Per-step chain-latency-optimized design. Total time = T * chain_latency
(the LSTM recurrence is serial in T; batch streams only add parallelism
until an engine saturates), so everything aims at the chain:

  - bf16 matmuls (1 cyc/row vs 4 for fp32) and bf16 DVE tiles.
  - v1 gate encoding: chunk0 = [i;f] -> Sigmoid, chunk1 = [g;o] -> Tanh
    with o-rows of W,b pre-scaled by 0.5 (tanh gives yt = 2*sigmoid-1).
    This makes u = si*tg, v = sf*c, c' = u+v all plain tensor_tensor ops
    (bf16 2x_1p mode; scalar_tensor_tensor only runs 1x) — only
    h' = (yt+1)*tanh(c') needs the 1x STT.
  - Matmul emission order: all 3 psA (chunk0) matmuls, then psB, so the
    Sigmoid can start while chunk1 matmuls still run; u = si*tg waits for
    Tanh, v = sf*c only for Sigmoid (hides under Tanh).
  - PE heater: a few dependency-free matmuls after each step's real ones
    keep the PE HAM clock-gate at 8/8 (idle >3.4us re-throttles to 1.2GHz,
    doubling the in-chain matmul block).
  - x host-transposed to [T, I, BL]: per-step DMA is 6 contiguous 256B
    descriptors.

Layout per core (BL=256 split into 2 streams of BS=128), state transposed
[H, batch], layers fused along free dim (cols 0:BS = L1 at t=k, BS:2BS =
L2 at t=k-1). RR tile [71, 2BS]: rows 0:64 h' (=2h), row 64 ones, rows
65:71 x^T. Matmuls per 128-gate-row chunk:
  L1: K=71 fused [Whh0 | b | Wih0] @ [h1'; 1; x]
  L2: K=64 Whh1 @ h2' + K=65 [Wih1 | b] @ [h1'; 1]  (PSUM accumulate)
"""

import numpy as np

H = 64
I = 6
O = 6
NCORES = 8
N_HEAT = 6  # heater matmuls per stream per step (hang off S tile)


def _build(nc, tc, BL, BS, T, dt):
    import concourse.bass as bass
    from concourse import mybir

    f32 = mybir.dt.float32
    AF = mybir.ActivationFunctionType
    OP = mybir.AluOpType
    nstreams = BL // BS

    x_d = nc.dram_tensor("x", [T, I, BL], dt, kind="ExternalInput")
    w1_d = nc.dram_tensor("w1", [128, 256], dt, kind="ExternalInput")
    w2a_d = nc.dram_tensor("w2a", [128, 256], dt, kind="ExternalInput")
    w2b_d = nc.dram_tensor("w2b", [128, 256], dt, kind="ExternalInput")
    wl_d = nc.dram_tensor("wl", [65, O], dt, kind="ExternalInput")
    y_d = nc.dram_tensor("y", [BL, O], f32, kind="ExternalOutput")

    yT = y_d[:, :].rearrange("b o -> o b")                 # [O, BL]

    import contextlib
    ctx = contextlib.ExitStack()
    wp = ctx.enter_context(tc.tile_pool(name="w", bufs=1))
    rrp = ctx.enter_context(tc.tile_pool(name="rr", bufs=3))
    cp = ctx.enter_context(tc.tile_pool(name="c", bufs=2))
    sp = ctx.enter_context(tc.tile_pool(name="s", bufs=3))
    pp = ctx.enter_context(tc.tile_pool(name="ps", bufs=2, space="PSUM"))
    pfp = ctx.enter_context(tc.tile_pool(name="psf", bufs=1, space="PSUM"))

    # --- weights to SBUF ---
    w1c = []
    w2ac = []
    w2bc = []
    for c in range(2):
        t_ = wp.tile([128, 128], dt, tag=f"w1c{c}")
        nc.sync.dma_start(out=t_, in_=w1_d[:, c * 128:(c + 1) * 128])
        w1c.append(t_)
        t_ = wp.tile([128, 128], dt, tag=f"w2a{c}")
        nc.sync.dma_start(out=t_, in_=w2a_d[:, c * 128:(c + 1) * 128])
        w2ac.append(t_)
        t_ = wp.tile([128, 128], dt, tag=f"w2b{c}")
        nc.sync.dma_start(out=t_, in_=w2b_d[:, c * 128:(c + 1) * 128])
        w2bc.append(t_)
    wl = wp.tile([65, O], dt, tag="wl")
    nc.sync.dma_start(out=wl, in_=wl_d[:, :])

    # One-time PE warmup burst: ~40 back-to-back matmuls give the HAM
    # clock-gate its 3.4us of sustained busy so it opens to 2.4GHz before
    # the recurrence starts. Per-tick heaters (below, reading chain tiles
    # so they cannot run ahead of their tick) keep it warm after that:
    # the HAM MID window re-throttles on mostly-idle 3.4us windows, so the
    # PE-idle span of each tick must stay covered with activity.
    heat_ps = pfp.tile([128, 2 * BS], f32, tag="heat")
    for _ in range(40):
        nc.tensor.matmul(heat_ps[:, 0:128], w2ac[0], w2ac[0][0:128, 0:128],
                         start=True, stop=True)

    # persistent ring tiles, per stream
    rrs = []
    csts = []
    for s in range(nstreams):
        bs0 = s * BS
        rr = [rrp.tile([128, 2 * BS], dt, tag=f"rr{s}", name=f"rr{s}_{j}") for j in range(3)]
        cst = [cp.tile([128, 2 * BS], dt, tag=f"c{s}", name=f"c{s}_{j}") for j in range(2)]
        for t_ in rr:
            nc.vector.memset(t_[0:64, :], 0.0)
            nc.vector.memset(t_[64:128, :], 0.0)
            nc.vector.memset(t_[64:65, :], 1.0)
        for t_ in cst:
            nc.vector.memset(t_[64:128, :], 0.0)
        # x for tick 0
        nc.sync.dma_start(out=rr[0][65:71, 0:BS], in_=x_d[0, :, bs0:bs0 + BS])
        rrs.append(rr)
        csts.append(cst)

    # Anti-phase forcing: delay stream1's first cell update until stream0's
    # first one has completed, so the two chains run ~half a period apart.
    # Offset chains keep the scalar engine busy when the other chain's sem
    # fires, hiding its ~0.5us idle-dispatch latency. (Writes zeros — the
    # same value the memset put there — so it's numerically a no-op.)
    if nstreams == 2:
        nc.vector.tensor_scalar(
            out=csts[1][1][64:128, 0:BS], in0=csts[0][1][64:128, 0:BS],
            scalar1=0.0, scalar2=None, op0=mybir.AluOpType.mult)

    import concourse.tile as tile_mod

    for k in range(T + 1):
        prev_tanh_inst = None
        for s in range(nstreams):
            bs0 = s * BS
            rr = rrs[s]
            cst = csts[s]
            do1 = k < T
            do2 = k > 0
            a, b = (0, 2 * BS) if (do1 and do2) else ((0, BS) if do1 else (BS, 2 * BS))
            rcur = rr[k % 3]
            rnxt = rr[(k + 1) % 3]
            ccur = cst[k % 2]
            cnxt = cst[(k + 1) % 2]
            A = slice(0, BS)
            Bc = slice(BS, 2 * BS)
            cs = slice(a, b)

            psG = pp.tile([128, 4 * BS], f32, tag=f"pG{s}")
            psA = psG[:, 0:2 * BS]
            psB = psG[:, 2 * BS:4 * BS]
            # all chunk0 (psA) matmuls first so Sigmoid starts ASAP
            if do1:
                nc.tensor.matmul(psA[:, A], w1c[0], rcur[0:128, A], start=True, stop=True)
            if do2:
                nc.tensor.matmul(psA[:, Bc], w2ac[0], rcur[0:128, Bc], start=True, stop=False)
                nc.tensor.matmul(psA[:, Bc], w2bc[0], rcur[0:128, A], start=False, stop=True)
            if do1:
                nc.tensor.matmul(psB[:, A], w1c[1], rcur[0:128, A], start=True, stop=True)
            if do2:
                nc.tensor.matmul(psB[:, Bc], w2ac[1], rcur[0:128, Bc], start=True, stop=False)
                nc.tensor.matmul(psB[:, Bc], w2bc[1], rcur[0:128, A], start=False, stop=True)

            S = sp.tile([128, 2 * BS], dt, tag=f"S{s}")
            TY = sp.tile([128, 2 * BS], dt, tag=f"TY{s}")
            sig_i = nc.scalar.activation(S[:, cs], psA[:, cs], AF.Sigmoid)
            # Force ACT order SIG0,TANH0,SIG1,...: the scheduler's cost
            # model (HAM-cold matmul times) wrongly thinks psB isn't ready
            # and slots SIG1 between SIG0 and TANH0, putting ~440ns of s1
            # work onto s0's critical chain.
            if prev_tanh_inst is not None:
                tile_mod.add_dep_helper(
                    sig_i.ins, prev_tanh_inst, sync=False,
                    reason="force ACT order: s1 SIG after s0 TANH")
            # Heaters hang off S (the EARLIEST chain tile of this tick):
            # they become ready mid-chain and drain well before the next
            # tick's real matmuls are ready, so they fill PE-idle time
            # without ever delaying the chain.
            for _ in range(N_HEAT):
                nc.tensor.matmul(heat_ps[:, :], w2ac[0], S[0:128, :],
                                 start=True, stop=True)
            tanh_i = nc.scalar.activation(TY[:, cs], psB[:, cs], AF.Tanh)
            prev_tanh_inst = tanh_i.ins

            u = sp.tile([64, 2 * BS], dt, tag=f"u{s}")
            v = sp.tile([64, 2 * BS], dt, tag=f"v{s}")
            # v = sigmoid(f) * c       (base 64; only needs S -> hides under TY)
            nc.vector.tensor_tensor(out=v[:, cs], in0=S[64:128, cs],
                                    in1=ccur[64:128, cs], op=OP.mult)
            # u = sigmoid(i) * tanh(g) (base 0)
            nc.vector.tensor_tensor(out=u[:, cs], in0=S[0:64, cs],
                                    in1=TY[0:64, cs], op=OP.mult)
            # c' = u + v -> rows 64:128 of cnxt
            nc.vector.tensor_tensor(out=cnxt[64:128, cs], in0=u[:, cs],
                                    in1=v[:, cs], op=OP.add)
            # w = yt + 1 (off-chain, 2x-rate tensor_scalar; fills DVE idle)
            w_ = sp.tile([128, 2 * BS], dt, tag=f"w{s}")
            nc.vector.tensor_scalar(
                out=w_[64:128, cs], in0=TY[64:128, cs], scalar1=1.0,
                scalar2=None, op0=OP.add)
            # TC = tanh(c') at base 64
            TC = sp.tile([128, 2 * BS], dt, tag=f"TC{s}")
            nc.scalar.activation(TC[64:128, cs], cnxt[64:128, cs], AF.Tanh)
            # h' = w * TC -> rows 0:64 of rnxt (2x-rate TT on the tail)
            nc.vector.tensor_tensor(
                out=rnxt[0:64, cs], in0=w_[64:128, cs],
                in1=TC[64:128, cs], op=OP.mult)

            if k + 1 < T:
                nc.sync.dma_start(out=rnxt[65:71, 0:BS],
                                  in_=x_d[k + 1, :, bs0:bs0 + BS])

    # final linear per stream: y = [0.5*Wlin | blin] @ [h2'; 1]
    for s in range(nstreams):
        bs0 = s * BS
        rfin = rrs[s][(T + 1) % 3]
        psF = pfp.tile([O, BS], f32, tag=f"pF{s}")
        nc.tensor.matmul(psF[:, :], wl, rfin[0:65, BS:2 * BS], start=True, stop=True)
        oF = sp.tile([O, BS], f32, tag=f"oF{s}")
        nc.vector.tensor_copy(oF[:, :], psF[:, :])
        nc.sync.dma_start(out=yT[:, bs0:bs0 + BS], in_=oF)

    ctx.close()


def build_nc(BL=256, BS=128, T=512, dtype="bfloat16"):
    import concourse.bacc as bacc
    import concourse.tile as tile
    from concourse import mybir

    dt = getattr(mybir.dt, dtype)
    nc = bacc.Bacc(None, target_bir_lowering=False)
    with tile.TileContext(nc) as tc:
        _build(nc, tc, BL, BS, T, dt)
    nc.compile()
    return nc


def prep_weights(Wih0, Whh0, bih0, bhh0, Wih1, Whh1, bih1, bhh1, Wlin, blin,
                 np_dt):
    """Host-side weight prep (v1 encoding: only o-gate rows 0.5-scaled)."""
    f = np.float32
    b0 = (bih0 + bhh0).astype(f)
    b1 = (bih1 + bhh1).astype(f)

    def oscale(M):  # scale o-gate rows (192:256) by 0.5
        M = M.copy()
        M[192:256] *= 0.5
        return M

    w1 = np.concatenate([Whh0 * 0.5, b0[:, None], Wih0], axis=1).astype(f)
    w1 = oscale(w1)
    w2a = oscale((Whh1 * 0.5).astype(f))
    w2b = oscale(np.concatenate([Wih1 * 0.5, b1[:, None]], axis=1).astype(f))
    wlin_aug = np.concatenate([Wlin * 0.5, blin[:, None]], axis=1).astype(f)

    def pad128(M):  # [256, K] -> [128, 256] transposed with zero pad rows
        Mt = np.ascontiguousarray(M.T)
        out = np.zeros((128, Mt.shape[1]), dtype=Mt.dtype)
        out[:Mt.shape[0]] = Mt
        return out

    return {
        "w1": pad128(w1).astype(np_dt),                       # [128, 256]
        "w2a": pad128(w2a).astype(np_dt),                     # [128, 256]
        "w2b": pad128(w2b).astype(np_dt),                     # [128, 256]
        "wl": np.ascontiguousarray(wlin_aug.T).astype(np_dt), # [65, 6]
    }


_NC_CACHE = {}


def kernel(x, Wih0, Whh0, bih0, bhh0, Wih1, Whh1, bih1, bhh1, Wlin, blin,
           _trace=False):
    import ml_dtypes
    from concourse.bass_utils import run_bass_kernel_spmd

    np_dt = ml_dtypes.bfloat16
    x = np.ascontiguousarray(np.asarray(x, dtype=np.float32))
    B, T, _ = x.shape
    BL = B // NCORES
    key = (BL, T)
    if key not in _NC_CACHE:
        _NC_CACHE[key] = build_nc(BL=BL, BS=BL // 2, T=T)
    nc = _NC_CACHE[key]

    w = prep_weights(np.asarray(Wih0), np.asarray(Whh0), np.asarray(bih0),
                     np.asarray(bhh0), np.asarray(Wih1), np.asarray(Whh1),
                     np.asarray(bih1), np.asarray(bhh1), np.asarray(Wlin),
                     np.asarray(blin), np_dt)

    in_maps = []
    for c in range(NCORES):
        xc = x[c * BL:(c + 1) * BL]            # [BL, T, I]
        xt = np.ascontiguousarray(xc.transpose(1, 2, 0)).astype(np_dt)
        m = {"x": xt}
        m.update(w)
        in_maps.append(m)

    res = run_bass_kernel_spmd(nc, in_maps, core_ids=list(range(NCORES)),
                               trace=_trace)
    out = np.concatenate([r["y"] for r in res.results], axis=0)
    if _trace:
        kernel._last_result = res
    return out

